# revision 1
# baseline (speedup 1.0000x reference)
"""CrossMambaFusion Trainium2 kernel — 8-core SPMD via bass/Tile.

Sharding (hardcoded for B=2, C=256, H=W=64, Di=512, N=16, R=32, K=4):
  core c -> batch b = c//4, d_inner slice q = c%4 (128 channels).
  On-device layout is feature-major [features, tokens]; (B,C,H,W) inputs
  reshape to (C, L=4096) with no host transpose.
  Per core: full-L front (dec/enc proj, gating, in_proj, causal conv, x_proj,
  dt), selective scan over its 128 d-channels x 16 state dims via DVE
  tensor_tensor_scan, one 8-rank AllToAll of y, then the tail (m_out, out
  gate, out_proj, residual, layernorm) on its L-quarter.

  SPMD trick 1: xm/u channel order is permuted per core (own d-slice first)
  via host-permuted weights, so "my slice" is always k-tile 0.
  SPMD trick 2: 8-rank AllToAll with duplicated quarter-shards; the m_out
  weight rows belonging to the other batch group are zeroed per core, so the
  K=1024 contraction drops cross-batch contributions (static offsets, SPMD).
"""
import numpy as np
import ml_dtypes

bf16 = ml_dtypes.bfloat16

B, C, Hh, Ww = 2, 256, 64, 64
L = Hh * Ww
Di, N, R, KC = 512, 16, 32, 4
DQ = 128
LQ = L // 4
NCORES = 8
LH = L // 2

_cache = {}


def _build():
    import concourse.bass as bass
    import concourse.mybir as mybir
    import concourse.tile as tile
    from concourse import bacc

    fp32 = mybir.dt.float32
    bfl = mybir.dt.bfloat16
    AF = mybir.ActivationFunctionType
    OP = mybir.AluOpType
    ts = bass.ts

    nc = bacc.Bacc("TRN2", target_bir_lowering=False, num_devices=NCORES)

    def din(name, shape, dt=fp32):
        return nc.declare_dram_parameter(name, list(shape), dt, isOutput=False)

    dec_bf = din("dec_bf", (C, L), bfl)
    enc_bf = din("enc_bf", (C, L), bfl)
    dec_f32q = din("dec_f32q", (C, LQ), fp32)
    w_dec_x = din("w_dec_x", (C, Di), bfl)
    w_dec_g = din("w_dec_g", (C, Di), bfl)
    b_dec_x = din("b_dec_x", (Di, 1))
    b_dec_g = din("b_dec_g", (Di, 1))
    w_enc = din("w_enc", (C, Di), bfl)
    b_enc = din("b_enc", (Di, 1))
    w_in_x = din("w_in_x", (Di, Di), bfl)      # columns permuted (own slice first)
    b_in_x = din("b_in_x", (Di, 1))            # permuted
    w_in_z = din("w_in_z", (Di, DQ), bfl)
    b_in_z = din("b_in_z", (DQ, 1))
    conv_w4 = din("conv_w4", (Di, KC))         # permuted rows
    conv_b = din("conv_b", (Di, 1))            # permuted
    w_xp = din("w_xp", (Di, 2 * R), bfl)       # permuted rows
    w_dt = din("w_dt", (R, DQ), bfl)
    b_dt = din("b_dt", (DQ, 1))
    a_sl = din("a_sl", (DQ, N))
    d_col = din("d_col", (DQ, 1))
    w_mo = din("w_mo", (2 * Di, Di), bfl)      # rows of other batch group zeroed
    b_mo = din("b_mo", (Di, 1))
    w_out = din("w_out", (Di, C), bfl)
    b_out = din("b_out", (C, 1))
    g_col = din("g_col", (C, 1))
    bln_col = din("bln_col", (C, 1))

    res_out = nc.declare_dram_parameter("res", [C, LQ], fp32, isOutput=True)

    LC = 512
    NL = L // LC

    with tile.TileContext(nc) as tc:
        import contextlib
        with contextlib.ExitStack() as stack:
            wpool = stack.enter_context(tc.tile_pool(name="weights", bufs=1))
            cpool = stack.enter_context(tc.tile_pool(name="consts", bufs=1))
            dpool = stack.enter_context(tc.tile_pool(name="drambuf", bufs=1, space="DRAM"))

            bc_rows = dpool.tile([2 * N, L], bfl)     # B rows then C rows
            ln_rows = dpool.tile([2, LQ], fp32)       # mu, rstd
            a2a_in = dpool.tile([2 * Di, LQ], bfl)
            a2a_out = dpool.tile([2 * Di, LQ], bfl)

            def wload(ap, kt, m, name):
                t = wpool.tile([128, kt, m], bfl, tag=name, name=name)
                nc.sync.dma_start(out=t[:], in_=ap.ap().rearrange("(t k) m -> k t m", k=128))
                return t

            sw_dec_x = wload(w_dec_x, 2, Di, "w_dec_x")
            sw_enc = wload(w_enc, 2, Di, "w_enc")

            def cload(ap, nt, name, cols=1):
                if nt == 1:
                    t = cpool.tile([128, cols], fp32, tag=name, name=name)
                    nc.sync.dma_start(out=t[:], in_=ap.ap())
                else:
                    t = cpool.tile([128, nt, cols], fp32, tag=name, name=name)
                    nc.sync.dma_start(out=t[:], in_=ap.ap().rearrange("(t k) o -> k t o", k=128))
                return t

            sb_dec_x = cload(b_dec_x, 4, "b_dec_x")
            sb_dec_g = cload(b_dec_g, 4, "b_dec_g")
            sb_enc = cload(b_enc, 4, "b_enc")
            sb_in_x = cload(b_in_x, 4, "b_in_x")
            sb_in_z = cload(b_in_z, 1, "b_in_z")
            s_convw = cload(conv_w4, 4, "conv_w4", cols=KC)
            s_convb = cload(conv_b, 4, "conv_b")
            sb_dt = cload(b_dt, 1, "b_dt")
            s_a = cload(a_sl, 1, "a_sl", cols=N)
            s_d = cload(d_col, 1, "d_col")
            sb_mo = cload(b_mo, 4, "b_mo")
            sb_out = cload(b_out, 2, "b_out")
            s_g = cload(g_col, 2, "g_col")
            s_bln = cload(bln_col, 2, "bln_col")

            # persistent across front -> scan -> a2a
            spool0 = stack.enter_context(tc.tile_pool(name="scanin", bufs=1))
            s_dt = spool0.tile([128, L], fp32)
            s_dtu = spool0.tile([128, L], bfl)
            s_uD = spool0.tile([128, L], bfl)
            s_siluz = spool0.tile([128, L], bfl)
            s_y = spool0.tile([128, L], bfl)

            sw_in_x = wload(w_in_x, 4, Di, "w_in_x")
            sw_in_z = wload(w_in_z, 4, DQ, "w_in_z")
            sw_xp = wload(w_xp, 4, 2 * R, "w_xp")
            sw_mo = wload(w_mo, 8, Di, "w_mo")
            sw_out = wload(w_out, 4, C, "w_out")
            sw_dec_g = wload(w_dec_g, 2, Di, "w_dec_g")
            sw_dt = wpool.tile([R, DQ], bfl)
            nc.sync.dma_start(out=sw_dt[:], in_=w_dt.ap())

            with tc.tile_pool(name="ph12", bufs=1) as ppool:
                s_comb = ppool.tile([128, 4, L], bfl)
                s_u = ppool.tile([128, 4, L], bfl)

                # ---- phase 1: combined = dec_x*sig(enc_p) + enc_p ----
                with tc.tile_pool(name="ph1", bufs=1) as f1pool, \
                     tc.tile_pool(name="ph1c", bufs=3) as f1c, \
                     tc.tile_pool(name="ps1", bufs=3, space="PSUM") as ps1:
                    s_dec = f1pool.tile([128, 2, L], bfl)
                    s_enc = f1pool.tile([128, 2, L], bfl)
                    dec_r = dec_bf.ap().rearrange("(t k) l -> k t l", k=128)
                    enc_r = enc_bf.ap().rearrange("(t k) l -> k t l", k=128)
                    for pc in range(4):
                        pl = ts(pc, LQ)
                        nc.sync.dma_start(out=s_dec[:, :, pl], in_=dec_r[:, :, pl])
                        nc.sync.dma_start(out=s_enc[:, :, pl], in_=enc_r[:, :, pl])
                    for lc in range(NL):
                        ls = ts(lc, LC)
                        for m in range(4):
                            ps_dx = ps1.tile([128, LC], fp32, tag="ps_dx", name="ps_dx")
                            ps_ep = ps1.tile([128, LC], fp32, tag="ps_ep", name="ps_ep")
                            for t in range(2):
                                nc.tensor.matmul(ps_dx[:], sw_dec_x[:, t, ts(m, 128)],
                                                 s_dec[:, t, ls], start=(t == 0), stop=(t == 1))
                            for t in range(2):
                                nc.tensor.matmul(ps_ep[:], sw_enc[:, t, ts(m, 128)],
                                                 s_enc[:, t, ls], start=(t == 0), stop=(t == 1))
                            sg = f1c.tile([128, LC], bfl, tag="sg", name="sg")
                            nc.scalar.activation(sg[:], ps_ep[:], AF.Sigmoid,
                                                 bias=sb_enc[:, m, :])
                            tm = f1c.tile([128, LC], bfl, tag="tm", name="tm")
                            nc.vector.scalar_tensor_tensor(tm[:], ps_dx[:],
                                                           sb_dec_x[:, m, :], sg[:],
                                                           OP.add, OP.mult)
                            nc.vector.scalar_tensor_tensor(s_comb[:, m, ls], ps_ep[:],
                                                           sb_enc[:, m, :], tm[:],
                                                           OP.add, OP.add)

                # ---- phase 2: in_proj/conv/silu/x_proj/dt ----
                with tc.tile_pool(name="ph2", bufs=1) as m1pool, \
                     tc.tile_pool(name="ph2c", bufs=2) as m2c, \
                     tc.tile_pool(name="ps2x", bufs=3, space="PSUM") as ps2x, \
                     tc.tile_pool(name="ps2", bufs=2, space="PSUM") as ps2:
                    s_xm = m1pool.tile([128, 4, 3 + L], bfl)
                    for m in range(4):
                        nc.vector.memset(s_xm[:, m, 0:3], 0.0)
                    for lc in range(NL):
                        ls = ts(lc, LC)
                        for m in range(4):
                            ps_xm = ps2x.tile([128, LC], fp32, tag="ps_xm", name="ps_xm")
                            for t in range(4):
                                nc.tensor.matmul(ps_xm[:], sw_in_x[:, t, ts(m, 128)],
                                                 s_comb[:, t, ls], start=(t == 0), stop=(t == 3))
                            nc.scalar.activation(s_xm[:, m, 3 + lc * LC:3 + (lc + 1) * LC],
                                                 ps_xm[:], AF.Identity, bias=sb_in_x[:, m, :])
                        ps_z = ps2.tile([128, LC], fp32, tag="ps_z", name="ps_z")
                        for t in range(4):
                            nc.tensor.matmul(ps_z[:], sw_in_z[:, t, :], s_comb[:, t, ls],
                                             start=(t == 0), stop=(t == 3))
                        nc.scalar.activation(s_siluz[:, ls], ps_z[:], AF.Silu,
                                             bias=sb_in_z[:, 0:1])

                    # causal depthwise conv: tensor_scalar + 3 stt chains on DVE
                    for m in range(4):
                        for piece in range(4):
                            o = piece * LQ
                            acc = m2c.tile([128, LQ], fp32, tag=f"acc{m % 2}", name="acc")
                            nc.vector.tensor_scalar(acc[:], s_xm[:, m, o:o + LQ],
                                                    s_convw[:, m, 0:1], None, OP.mult)
                            for k in range(1, KC):
                                nc.vector.scalar_tensor_tensor(
                                    acc[:], s_xm[:, m, o + k:o + k + LQ],
                                    s_convw[:, m, k:k + 1], acc[:], OP.mult, OP.add)
                            nc.scalar.activation(s_u[:, m, o:o + LQ], acc[:], AF.Silu,
                                                 bias=s_convb[:, m, :])
                    # x_proj + B/C row spill + dt
                    for lc in range(NL):
                        ls = ts(lc, LC)
                        ps_xd = ps2.tile([128, LC], fp32, tag="xddt", name="ps_xd")
                        for t in range(4):
                            nc.tensor.matmul(ps_xd[0:64, :], sw_xp[:, t, :], s_u[:, t, ls],
                                             start=(t == 0), stop=(t == 3))
                        dtin = m2c.tile([R, LC], bfl, tag="dtin", name="dtin")
                        nc.scalar.activation(dtin[:], ps_xd[0:R, :], AF.Copy)
                        bcl = m2c.tile([64, LC], bfl, tag="bcl", name="bcl")
                        nc.vector.tensor_copy(bcl[32:64, :], ps_xd[32:64, :])
                        nc.sync.dma_start(out=bc_rows[:, ls], in_=bcl[32:64, :])
                        ps_dt = ps2.tile([128, LC], fp32, tag="xddt", name="ps_dt")
                        nc.tensor.matmul(ps_dt[:], sw_dt[:, :], dtin[:],
                                         start=True, stop=True)
                        # softplus(x) = relu(x) + ln(1 + exp(-|x|)); x = psum + dt_b
                        ab = m2c.tile([128, LC], fp32, tag="sp1", name="ab")
                        nc.scalar.activation(ab[:], ps_dt[:], AF.Abs, bias=sb_dt[:, 0:1])
                        ex = m2c.tile([128, LC], fp32, tag="sp2", name="ex")
                        nc.scalar.activation(ex[:], ab[:], AF.Exp, scale=-1.0)
                        ln1 = m2c.tile([128, LC], fp32, tag="sp1", name="ln1")
                        nc.scalar.activation(ln1[:], ex[:], AF.Ln, bias=1.0)
                        rl = m2c.tile([128, LC], fp32, tag="sp2", name="rl")
                        nc.scalar.activation(rl[:], ps_dt[:], AF.Relu, bias=sb_dt[:, 0:1])
                        nc.vector.tensor_tensor(s_dt[:, ls], rl[:], ln1[:], OP.add)

                    # dtu = dt * u_own ; uD = u_own * D  (halves -> scan overlap)
                    for half in range(2):
                        hs = ts(half, LH)
                        nc.vector.tensor_tensor(s_dtu[:, hs], s_dt[:, hs],
                                                s_u[:, 0, hs], OP.mult)
                        nc.vector.tensor_scalar(s_uD[:, hs], s_u[:, 0, hs],
                                                s_d[:, 0:1], None, OP.mult)

            # ---- pre-tail: decoder gate path (overlaps the scan on PE/ACT) ----
            ptpool = stack.enter_context(tc.tile_pool(name="pretail", bufs=1))
            s_decf = ptpool.tile([128, 2, LQ], fp32)
            s_decq = ptpool.tile([128, 2, LQ], bfl)
            s_sgate = ptpool.tile([128, 4, LQ], bfl)
            nc.sync.dma_start(out=s_decf[:],
                              in_=dec_f32q.ap().rearrange("(t k) l -> k t l", k=128))
            nc.vector.tensor_copy(s_decq[:], s_decf[:])
            with tc.tile_pool(name="psg", bufs=2, space="PSUM") as psg:
                for lc in range(LQ // LC):
                    ls = ts(lc, LC)
                    for m in range(4):
                        ps_g = psg.tile([128, LC], fp32, tag="ps_g", name="ps_g")
                        for t in range(2):
                            nc.tensor.matmul(ps_g[:], sw_dec_g[:, t, ts(m, 128)],
                                             s_decq[:, t, ls], start=(t == 0), stop=(t == 1))
                        nc.scalar.activation(s_sgate[:, m, ls], ps_g[:], AF.Sigmoid,
                                             bias=sb_dec_g[:, m, :])

            # ---- phase 3: selective scan ----
            bca = bc_rows[0:1, 0:1]
            with tc.tile_pool(name="scan", bufs=2) as spool, \
                 tc.tile_pool(name="scan1", bufs=1) as s1pool:
                ysum = s1pool.tile([128, L], bfl)
                for n in range(N):
                    w = n % 2
                    h = spool.tile([128, L], bfl, tag=f"h{w}", name=f"h{w}")
                    for half in range(2):
                        hs = ts(half, LH)
                        a = spool.tile([128, LH], bfl, tag=f"a{w}", name=f"a{w}")
                        bt = spool.tile([128, LH], bfl, tag=f"b{w}", name=f"b{w}")
                        bbc = spool.tile([128, LH], bfl, tag=f"bbc{w}", name=f"bbc{w}")
                        nc.sync.dma_start(out=bbc[:], in_=bass.AP(
                            tensor=bca.tensor, offset=bca.offset + n * L + half * LH,
                            ap=[[0, 128], [1, LH]]))
                        nc.scalar.activation(a[:], s_dt[:, hs], AF.Exp,
                                             scale=s_a[:, n:n + 1])
                        nc.vector.tensor_tensor(bt[:], s_dtu[:, hs], bbc[:], OP.mult)
                        init = 0.0 if half == 0 else h[:, LH - 1:LH]
                        nc.vector.tensor_tensor_scan(h[:, hs], a[:], bt[:], init,
                                                     OP.mult, OP.add)
                    for half in range(2):
                        hs = ts(half, LH)
                        cbc = spool.tile([128, LH], bfl, tag=f"cbc{w}", name=f"cbc{w}")
                        nc.sync.dma_start(out=cbc[:], in_=bass.AP(
                            tensor=bca.tensor, offset=bca.offset + (N + n) * L + half * LH,
                            ap=[[0, 128], [1, LH]]))
                        nc.vector.tensor_tensor(h[:, hs], h[:, hs], cbc[:], OP.mult)
                    if n == 0:
                        nc.vector.tensor_tensor(ysum[:], h[:], s_uD[:], OP.add)
                    else:
                        nc.vector.tensor_tensor(ysum[:], ysum[:], h[:], OP.add)
                nc.vector.tensor_tensor(s_y[:], ysum[:], s_siluz[:], OP.mult)

            # ---- phase 4: 8-rank AllToAll (shard j = own y quarter j%4) ----
            for j in range(8):
                nc.sync.dma_start(out=a2a_in[j * 128:(j + 1) * 128, :],
                                  in_=s_y[:, ts(j % 4, LQ)])
            nc.gpsimd.collective_compute(
                "AllToAll", mybir.AluOpType.bypass,
                replica_groups=[[0, 1, 2, 3, 4, 5, 6, 7]],
                ins=[a2a_in[:, :]], outs=[a2a_out[:, :]],
            )

            # ---- phase 5: tail on own L-quarter ----
            with tc.tile_pool(name="tail", bufs=2) as tpool, \
                 tc.tile_pool(name="tail1", bufs=1) as t1pool, \
                 tc.tile_pool(name="ps3", bufs=2, space="PSUM") as ps3, \
                 tc.tile_pool(name="ps3s", bufs=1, space="PSUM") as ps3s:
                s_yall = t1pool.tile([128, 8, LQ], bfl)
                a2a_r = a2a_out[:, :].rearrange("(t k) l -> k t l", k=128)
                for pc in range(2):
                    pl = ts(pc, LC)
                    nc.sync.dma_start(out=s_yall[:, :, pl], in_=a2a_r[:, :, pl])
                s_res = t1pool.tile([128, 2, LQ], fp32)
                s_res2 = t1pool.tile([128, 2, LQ], fp32)
                ones = t1pool.tile([128, 1], fp32)
                nc.vector.memset(ones[:], 1.0)

                s_gated = t1pool.tile([128, 4, LQ], bfl)
                NLQ = LQ // LC
                ps_sum = [ps3s.tile([1, LC], fp32, tag=f"ps_sum{lc}", name=f"ps_sum{lc}")
                          for lc in range(NLQ)]
                ps_sq = [ps3s.tile([1, LC], fp32, tag=f"ps_sq{lc}", name=f"ps_sq{lc}")
                         for lc in range(NLQ)]
                for lc in range(NLQ):
                    ls = ts(lc, LC)
                    for m in range(4):
                        ps_mo = ps3.tile([128, LC], fp32, tag="mm3", name="ps_mo")
                        for t in range(8):
                            nc.tensor.matmul(ps_mo[:], sw_mo[:, t, ts(m, 128)],
                                             s_yall[:, t, ls], start=(t == 0), stop=(t == 7))
                        spr = tpool.tile([128, LC], bfl, tag="spr", name="spr")
                        nc.scalar.activation(spr[:], ps_mo[:], AF.Identity,
                                             bias=sb_mo[:, m, :])
                        nc.vector.tensor_tensor(s_gated[:, m, ls], spr[:],
                                                s_sgate[:, m, ls], OP.mult)
                    for m in range(2):
                        ps_o = ps3.tile([128, LC], fp32, tag="mm3", name="ps_o")
                        for t in range(4):
                            nc.tensor.matmul(ps_o[:], sw_out[:, t, ts(m, 128)],
                                             s_gated[:, t, ls], start=(t == 0), stop=(t == 3))
                        nc.vector.scalar_tensor_tensor(s_res[:, m, ls], ps_o[:],
                                                       sb_out[:, m, :], s_decf[:, m, ls],
                                                       OP.add, OP.add)
                        nc.scalar.activation(s_res2[:, m, ls], s_res[:, m, ls], AF.Square)
                        nc.tensor.matmul(ps_sum[lc][:], ones[:], s_res[:, m, ls],
                                         start=(m == 0), stop=(m == 1))
                        nc.tensor.matmul(ps_sq[lc][:], ones[:], s_res2[:, m, ls],
                                         start=(m == 0), stop=(m == 1))

                mu = t1pool.tile([1, LQ], fp32)
                musq = t1pool.tile([1, LQ], fp32)
                var = t1pool.tile([1, LQ], fp32)
                sd = t1pool.tile([1, LQ], fp32)
                rstd = t1pool.tile([1, LQ], fp32)
                for lc in range(NLQ):
                    ls = ts(lc, LC)
                    nc.scalar.activation(mu[:, ls], ps_sum[lc][:], AF.Copy, scale=1.0 / C)
                    nc.scalar.activation(musq[:, ls], mu[:, ls], AF.Square)
                    nc.vector.scalar_tensor_tensor(var[:, ls], ps_sq[lc][:], 1.0 / C,
                                                   musq[:, ls], OP.mult, OP.subtract)
                eps = t1pool.tile([1, 1], fp32)
                nc.vector.memset(eps[:], 1e-5)
                nc.scalar.activation(sd[:], var[:], AF.Sqrt, bias=eps[:, 0:1])
                nc.vector.reciprocal(rstd[:], sd[:])
                nc.sync.dma_start(out=ln_rows[0:1, :], in_=mu[:])
                nc.sync.dma_start(out=ln_rows[1:2, :], in_=rstd[:])
                mu_bc = t1pool.tile([128, LQ], fp32)
                rs_bc = t1pool.tile([128, LQ], fp32)
                lna = ln_rows[0:1, 0:1]
                nc.sync.dma_start(out=mu_bc[:], in_=bass.AP(
                    tensor=lna.tensor, offset=lna.offset, ap=[[0, 128], [1, LQ]]))
                nc.sync.dma_start(out=rs_bc[:], in_=bass.AP(
                    tensor=lna.tensor, offset=lna.offset + LQ, ap=[[0, 128], [1, LQ]]))
                for m in range(2):
                    t1 = tpool.tile([128, LQ], fp32, tag="t1", name="t1")
                    nc.vector.tensor_tensor(t1[:], s_res[:, m, :], mu_bc[:], OP.subtract)
                    nc.vector.tensor_tensor(t1[:], t1[:], rs_bc[:], OP.mult)
                    nc.scalar.activation(t1[:], t1[:], AF.Identity,
                                         scale=s_g[:, m, :], bias=s_bln[:, m, :])
                    nc.sync.dma_start(
                        out=res_out.ap().rearrange("(t k) l -> k t l", k=128)[:, m, :],
                        in_=t1[:])

    nc.compile()
    return nc


def _in_maps(inp):
    A = -np.exp(inp["A_log"].astype(np.float32))
    dec_T = inp["decoder_feat"].reshape(B, C, L)
    enc_T = inp["encoder_feat"].reshape(B, C, L)
    dec_T_bf = dec_T.astype(bf16)
    enc_T_bf = enc_T.astype(bf16)

    def col(x):
        return np.ascontiguousarray(np.asarray(x, np.float32).reshape(-1, 1))

    common = {
        "w_dec_x": np.ascontiguousarray(inp["dec_w"][:, :Di].astype(bf16)),
        "w_dec_g": np.ascontiguousarray(inp["dec_w"][:, Di:].astype(bf16)),
        "b_dec_x": col(inp["dec_b"][:Di]),
        "b_dec_g": col(inp["dec_b"][Di:]),
        "w_enc": inp["enc_w"].astype(bf16),
        "b_enc": col(inp["enc_b"]),
        "b_mo": col(inp["m_out_b"]),
        "w_out": inp["out_w"].astype(bf16),
        "b_out": col(inp["out_b"]),
        "g_col": col(inp["ln_g"]),
        "bln_col": col(inp["ln_b"]),
    }

    in_maps = []
    for c in range(NCORES):
        b, q = c // 4, c % 4
        ds = slice(q * DQ, (q + 1) * DQ)
        perm = np.r_[np.arange(q * DQ, (q + 1) * DQ),
                     np.arange(0, q * DQ), np.arange((q + 1) * DQ, Di)]
        m = dict(common)
        m["dec_bf"] = dec_T_bf[b]
        m["enc_bf"] = enc_T_bf[b]
        m["dec_f32q"] = np.ascontiguousarray(dec_T[b][:, q * LQ:(q + 1) * LQ].astype(np.float32))
        m["w_in_x"] = np.ascontiguousarray(inp["in_w"][:, :Di][:, perm].astype(bf16))
        m["b_in_x"] = col(inp["in_b"][:Di][perm])
        m["w_in_z"] = np.ascontiguousarray(
            inp["in_w"][:, Di + q * DQ:Di + (q + 1) * DQ].astype(bf16))
        m["b_in_z"] = col(inp["in_b"][Di + q * DQ:Di + (q + 1) * DQ])
        m["conv_w4"] = np.ascontiguousarray(inp["conv_w"][perm, 0, :].astype(np.float32))
        m["conv_b"] = col(inp["conv_b"][perm])
        m["w_xp"] = np.ascontiguousarray(inp["x_proj_w"][perm, :].astype(bf16))
        m["w_dt"] = np.ascontiguousarray(inp["dt_w"][:, ds].astype(bf16))
        m["b_dt"] = col(inp["dt_b"][ds])
        wmo8 = np.zeros((2 * Di, Di), np.float32)
        for r in range(8):
            if r // 4 == b:
                rq = r % 4
                wmo8[r * DQ:(r + 1) * DQ] = inp["m_out_w"][rq * DQ:(rq + 1) * DQ]
        m["w_mo"] = wmo8.astype(bf16)
        m["a_sl"] = np.ascontiguousarray(A[ds])
        m["d_col"] = col(inp["D_param"][ds])
        in_maps.append(m)
    return in_maps


def kernel(**inputs):
    from concourse.bass_utils import run_bass_kernel_spmd

    inp = {k: np.asarray(v) for k, v in inputs.items()}
    if "nc" not in _cache:
        _cache["nc"] = _build()
    res = run_bass_kernel_spmd(_cache["nc"], _in_maps(inp), list(range(NCORES)))
    out = np.zeros((B, C, L), np.float32)
    for c in range(NCORES):
        b, q = c // 4, c % 4
        out[b][:, q * LQ:(q + 1) * LQ] = res.results[c]["res"]
    return out.reshape(B, C, Hh, Ww)


def run_traced(inp):
    from concourse.bass_utils import run_bass_kernel_spmd

    if "nc" not in _cache:
        _cache["nc"] = _build()
    return run_bass_kernel_spmd(_cache["nc"], _in_maps(inp), list(range(NCORES)),
                                trace=True)

